# revision 12
# baseline (speedup 1.0000x reference)
"""GAT (single-layer, multi-head) message-passing kernel for Trainium2.

Problem: nn_CongestionWrapperEncoder0 (gnn_message_passing).

  out[g,n,h,:] = sum_{e: dst(e)=n} softmax_e(lrelu(a_src[g,src]+a_dst[g,n])) * xh[g,src(e),h,:]
  with xh = emb[x[g]] @ W, a_src/a_dst head-wise inner products with att vectors.

Sharding: data-parallel over the G = B*DAYS = 16 graph axis, 2 graphs per
NeuronCore.  All per-edge/per-node float work runs on device; the host only
does integer index preprocessing (dst-sorting the shared edge list, padding,
and folding the tiny W/att_src/att_dst parameter products).

Device algorithm (per core, its 2 graphs "paired"):
  1. aall_T[0:4,j] = asrc_all  (emb @ v_src),  [4:8,j] = adst_all (emb @ v_dst)
     via PE matmuls against a host-supplied emb^T.
  2. T_base[j] = [emb[j](32) | asrc_all[j](4) | adst_all[j](4) | pad] (DRAM,
     256B rows); T_pair[s] = [T_base[x[g0,s]] | T_base[x[g1,s]]] (512B rows)
     and T_adst[s] = [adst(g0) | adst(g1)] (32B rows) via indirect gathers.
  3. Edges sorted by dst, node-tile (128 dst rows) aligned, chunked by 128.
     Per chunk: gather T_pair rows by src (both graphs in one 512B
     descriptor), gather T_adst rows by dst; p = max(exp(a), exp(0.2 a))
     (== exp(leakyrelu(a)) exactly); rhs = [p*feat | p]; one-hot(dst) matmul
     accumulates [nodes x (feat-agg | p-sum)] in PSUM.
  4. Per node tile: normalize by 1/(s+1e-16), transpose via PE, apply the
     block-diagonal W (so out = (sum w*feat) @ W = sum w*xh exactly), + bias.
"""

import os
import numpy as np

B, DAYS, N, E = 2, 8, 10000, 80000
C_IN, C_OUT, H = 32, 32, 4
NEG = 0.2
G_TOT = B * DAYS
NCORES = 8
GPC = G_TOT // NCORES  # graphs per core
P = 128
SB = 32  # chunks per gather batch (SB*128 edges per indirect DMA call)

F32 = None  # set lazily (mybir import inside functions keeps module import light)


def _prep_edges(adjacency):
    """Host-side integer preprocessing of the shared edge list.

    Returns the dst-sorted, node-tile-aligned, 128-padded chunk structure
    (identical for every graph/core since the edge list is shared).
    """
    src = np.concatenate([adjacency[0], np.arange(N)]).astype(np.int64)
    dst = np.concatenate([adjacency[1], np.arange(N)]).astype(np.int64)
    order = np.argsort(dst, kind="stable")
    src_s, dst_s = src[order], dst[order]
    # node tiles of 128 dst rows
    n_tiles = (N + P - 1) // P
    # edge range per tile via searchsorted
    bounds = np.searchsorted(dst_s, np.arange(0, (n_tiles + 1) * P, P))
    src_chunks, dstloc_chunks, dst_chunks = [], [], []
    tiles = []  # (tile_idx, n_lo, n_cnt, chunk_lo, n_chunks)
    chunk_cursor = 0
    for t in range(n_tiles):
        lo, hi = bounds[t], bounds[t + 1]
        cnt = hi - lo
        n_chunks = max(1, (cnt + P - 1) // P)
        pad = n_chunks * P - cnt
        s = np.concatenate([src_s[lo:hi], np.zeros(pad, np.int64)])
        d = np.concatenate([dst_s[lo:hi], np.zeros(pad, np.int64)])
        dl = np.concatenate(
            [dst_s[lo:hi] - t * P, np.full(pad, -1, np.int64)]
        )
        src_chunks.append(s.reshape(n_chunks, P))
        dst_chunks.append(d.reshape(n_chunks, P))
        dstloc_chunks.append(dl.reshape(n_chunks, P))
        n_lo = t * P
        tiles.append((t, n_lo, min(P, N - n_lo), chunk_cursor, n_chunks))
        chunk_cursor += n_chunks
    src_all = np.concatenate(src_chunks, 0)  # [NCH, 128]
    dst_all = np.concatenate(dst_chunks, 0)
    dstloc_all = np.concatenate(dstloc_chunks, 0)
    nch = src_all.shape[0]
    return {
        "tiles": tiles,
        "nch": nch,
        # [128, NCH]: partition p of chunk c holds edge (c, p)
        "src_idx": np.ascontiguousarray(src_all.T).astype(np.int32),
        "dstloc": np.ascontiguousarray(dstloc_all.T).astype(np.float32),
    }


def build_program(nch, tiles, trace_label="gat"):
    """Build the Bass/Tile program for one core (2 graphs)."""
    import concourse.bass as bass
    import concourse.bacc as bacc
    import concourse.mybir as mybir
    import concourse.tile as tile

    f32 = mybir.dt.float32
    i32 = mybir.dt.int32
    NPAD = ((N + P - 1) // P) * P  # 10112
    NB = NPAD // P  # 79

    nc = bacc.Bacc(
        "TRN2",
        target_bir_lowering=False,
        debug=False,
        enable_asserts=False,
        num_devices=NCORES,
    )

    # ---- external inputs ----
    emb_in = nc.dram_tensor("emb", [NPAD, C_IN], f32, kind="ExternalInput")
    embT_in = nc.dram_tensor("embT", [C_IN, NPAD], f32, kind="ExternalInput")
    vboth_in = nc.dram_tensor("vboth", [C_IN, 2 * H], f32, kind="ExternalInput")
    wbd_in = nc.dram_tensor("wbd", [P, P], f32, kind="ExternalInput")
    biasrep_in = nc.dram_tensor("biasrep", [P, P], f32, kind="ExternalInput")
    iota_in = nc.dram_tensor("iota", [P, P], f32, kind="ExternalInput")
    ident_in = nc.dram_tensor("ident", [P, P], f32, kind="ExternalInput")
    xg_in = nc.dram_tensor("xg", [P, GPC, NB], i32, kind="ExternalInput")
    srcidx_in = nc.dram_tensor("srcidx", [P, nch], i32, kind="ExternalInput")
    dstloc_in = nc.dram_tensor("dstloc", [P, nch], f32, kind="ExternalInput")
    out_dram = nc.dram_tensor(
        "out", [GPC, N, H * C_OUT], f32, kind="ExternalOutput"
    )

    with tile.TileContext(nc) as tc:
        with (
            tc.tile_pool(name="dramp", bufs=1, space="DRAM") as dramp,
            tc.tile_pool(name="constp", bufs=1) as constp,
        ):
            t_base = dramp.tile([NPAD, 64], f32)
            t_pair = dramp.tile([NPAD, 2 * 64], f32)

            # persistent SBUF constants
            wbd_sb = constp.tile([P, P], f32)
            biasrep_sb = constp.tile([P, P], f32)
            iota_sb = constp.tile([P, P], f32)
            ident_sb = constp.tile([P, P], f32)
            vboth_sb = constp.tile([C_IN, 2 * H], f32)
            embT_sb = constp.tile([C_IN, NPAD], f32)
            srcidx_sb = constp.tile([P, nch], i32)
            dstloc_sb = constp.tile([P, nch], f32)
            xg_sb = constp.tile([P, GPC, NB], i32)
            adst_sb = constp.tile([P, NB, 2 * H], f32)
            nc.sync.dma_start(out=wbd_sb[:], in_=wbd_in[:, :])
            nc.sync.dma_start(out=biasrep_sb[:], in_=biasrep_in[:, :])
            nc.sync.dma_start(out=iota_sb[:], in_=iota_in[:, :])
            nc.sync.dma_start(out=ident_sb[:], in_=ident_in[:, :])
            nc.sync.dma_start(out=vboth_sb[:], in_=vboth_in[:, :])
            nc.sync.dma_start(out=embT_sb[:], in_=embT_in[:, :])
            nc.sync.dma_start(out=srcidx_sb[:], in_=srcidx_in[:, :])
            nc.sync.dma_start(out=dstloc_sb[:], in_=dstloc_in[:, :])
            nc.sync.dma_start(out=xg_sb[:], in_=xg_in[:, :, :])

            # ---- phase 1: aall_T[8, NPAD] = [v_src|v_dst]^T @ emb^T ----
            aall_T = constp.tile([2 * H, NPAD], f32)
            with (
                tc.tile_pool(name="bpsum", bufs=2, space="PSUM") as bpsum,
            ):
                for c0 in range(0, NPAD, 512):
                    ch = min(512, NPAD - c0)
                    aps = bpsum.tile([2 * H, 512], f32, space="PSUM")
                    nc.tensor.matmul(
                        out=aps[:, 0:ch],
                        lhsT=vboth_sb[:],
                        rhs=embT_sb[:, c0 : c0 + ch],
                        start=True,
                        stop=True,
                    )
                    nc.vector.tensor_copy(
                        out=aall_T[:, c0 : c0 + ch], in_=aps[:, 0:ch]
                    )

            # ---- phase 2: build T_base rows [emb | asrc | adst | pad] ----
            with (
                tc.tile_pool(name="tbp", bufs=3) as tbp,
                tc.tile_pool(name="tbps", bufs=2, space="PSUM") as tbps,
            ):
                for c in range(NB):
                    tb = tbp.tile([P, 64], f32)
                    nc.vector.memset(tb[:], 0.0)
                    nc.sync.dma_start(
                        out=tb[:, 0:C_IN], in_=emb_in[c * P : (c + 1) * P, :]
                    )
                    atp = tbps.tile([P, 2 * H], f32, space="PSUM")
                    nc.tensor.transpose(
                        out=atp[:],
                        in_=aall_T[:, c * P : (c + 1) * P],
                        identity=ident_sb[: 2 * H, : 2 * H],
                    )
                    nc.vector.tensor_copy(out=tb[:, 32:40], in_=atp[:])
                    nc.sync.dma_start(
                        out=t_base[c * P : (c + 1) * P, :], in_=tb[:]
                    )

            # ---- phase 3: per-graph node gathers -> T_pair + SBUF adst ----
            with tc.tile_pool(name="gbp", bufs=4) as gbp:
                for i in range(NB):
                    pairt = gbp.tile([P, 2 * 64], f32, name="pairt")
                    for g in range(GPC):
                        gb = gbp.tile([P, 64], f32, name="gb")
                        nc.gpsimd.indirect_dma_start(
                            out=gb[:],
                            out_offset=None,
                            in_=t_base[:, :],
                            in_offset=bass.IndirectOffsetOnAxis(
                                ap=xg_sb[:, g, i : i + 1], axis=0
                            ),
                        )
                        nc.vector.tensor_copy(
                            out=pairt[:, 64 * g : 64 * (g + 1)], in_=gb[:]
                        )
                        nc.vector.tensor_copy(
                            out=adst_sb[:, i, 4 * g : 4 * (g + 1)],
                            in_=gb[:, 36:40],
                        )
                    nc.sync.dma_start(
                        out=t_pair[i * P : (i + 1) * P, :], in_=pairt[:]
                    )

            # ---- phase 4: main edge loop ----
            with (
                tc.tile_pool(name="edgep", bufs=8) as edgep,
                tc.tile_pool(name="rhsp", bufs=4) as rhsp,
                tc.tile_pool(name="ohp", bufs=4) as ohp,
                tc.tile_pool(name="smallp", bufs=6) as smallp,
                tc.tile_pool(name="aggp", bufs=2, space="PSUM") as aggp,
                tc.tile_pool(name="ohtpp", bufs=2, space="PSUM") as ohtpp,
                tc.tile_pool(name="adpp", bufs=2, space="PSUM") as adpp,
                tc.tile_pool(name="tpsp", bufs=1, space="PSUM") as tpsp,
                tc.tile_pool(name="outpsp", bufs=1, space="PSUM") as outpsp,
                tc.tile_pool(name="ntp", bufs=3) as ntp,
            ):
                for t, n_lo, n_cnt, chunk_lo, n_chunks in tiles:
                    agg = aggp.tile([P, 2 * 132], f32, space="PSUM")
                    for k in range(n_chunks):
                        c = chunk_lo + k
                        # gather this chunk's 128 src rows (both graphs/row)
                        ge = edgep.tile([P, 2 * 64], f32, name="ge")
                        nc.gpsimd.indirect_dma_start(
                            out=ge[:],
                            out_offset=None,
                            in_=t_pair[:, :],
                            in_offset=bass.IndirectOffsetOnAxis(
                                ap=srcidx_sb[:, c : c + 1], axis=0
                            ),
                        )
                        # one-hot of dstlocal, and its PE transpose
                        oh = ohp.tile([P, P], f32, name="oh")
                        nc.vector.tensor_scalar(
                            out=oh[:],
                            in0=iota_sb[:],
                            scalar1=dstloc_sb[:, c : c + 1],
                            scalar2=None,
                            op0=mybir.AluOpType.is_equal,
                        )
                        ohtp = ohtpp.tile([P, P], f32, space="PSUM")
                        nc.tensor.transpose(
                            out=ohtp[:], in_=oh[:], identity=ident_sb[:]
                        )
                        ohT = ohp.tile([P, P], f32, name="ohT")
                        nc.vector.tensor_copy(out=ohT[:], in_=ohtp[:])
                        # adst broadcast to edges: [128e, 8] = ohT.T @ adst_nt
                        adp = adpp.tile([P, 2 * H], f32, space="PSUM")
                        nc.tensor.matmul(
                            out=adp[:],
                            lhsT=ohT[:],
                            rhs=adst_sb[:, t, :],
                            start=True,
                            stop=True,
                        )
                        # alpha[p, g, h] = asrc(src row) + adst(dst row)
                        alpha = smallp.tile([P, 2, H], f32, name="alpha")
                        nc.vector.tensor_tensor(
                            out=alpha[:],
                            in0=ge[:, :]
                            .rearrange("p (g c) -> p g c", g=2)[:, :, 32:36],
                            in1=adp[:].rearrange("p (g h) -> p g h", g=2),
                            op=mybir.AluOpType.add,
                        )
                        e1 = smallp.tile([P, 2, H], f32, name="e1")
                        e2 = smallp.tile([P, 2, H], f32, name="e2")
                        nc.scalar.activation(
                            out=e1[:], in_=alpha[:],
                            func=mybir.ActivationFunctionType.Exp,
                        )
                        nc.scalar.activation(
                            out=e2[:], in_=alpha[:],
                            func=mybir.ActivationFunctionType.Exp,
                            scale=NEG,
                        )
                        rhs = rhsp.tile([P, 2, 132], f32, name="rhs")
                        # p = exp(lrelu(alpha)) -> rhs[:, g, 128:132]
                        nc.vector.tensor_tensor(
                            out=rhs[:, :, 128:132],
                            in0=e1[:],
                            in1=e2[:],
                            op=mybir.AluOpType.max,
                        )
                        # msgw = p * feat  -> rhs[:, g, 0:128] ([p,g,h,c] view)
                        nc.vector.tensor_tensor(
                            out=rhs[:, :, 0:128].rearrange(
                                "p g (h c) -> p g h c", h=H
                            ),
                            in0=ge[:, :]
                            .rearrange("p (g o c) -> p g o c", g=2, o=1)[
                                :, :, :, 0:32
                            ].to_broadcast([P, 2, H, 32]),
                            in1=rhs[:, :, 128:132]
                            .rearrange("p g (h o) -> p g h o", o=1)
                            .to_broadcast([P, 2, H, 32]),
                            op=mybir.AluOpType.mult,
                        )
                        nc.tensor.matmul(
                            out=agg[:],
                            lhsT=oh[:],
                            rhs=rhs[:],
                            start=(k == 0),
                            stop=(k == n_chunks - 1),
                        )

                    # ---- normalize + transform + bias + store ----
                    rs = smallp.tile([P, 2, H], f32, name="rs")
                    nc.vector.tensor_scalar(
                        out=rs[:],
                        in0=agg[:].rearrange("p (g c) -> p g c", g=2)[
                            :, :, 128:132
                        ],
                        scalar1=1e-16,
                        scalar2=None,
                        op0=mybir.AluOpType.add,
                    )
                    nc.vector.reciprocal(out=rs[:], in_=rs[:])
                    for g in range(GPC):
                        aggn = ntp.tile([P, P], f32, name="aggn")
                        nc.vector.tensor_tensor(
                            out=aggn[:].rearrange("p (h c) -> p h c", h=H),
                            in0=agg[:, 132 * g : 132 * g + 128].rearrange(
                                "p (h c) -> p h c", h=H
                            ),
                            in1=rs[:, g, :]
                            .rearrange("p (h o) -> p h o", o=1)
                            .to_broadcast([P, H, 32]),
                            op=mybir.AluOpType.mult,
                        )
                        tps = tpsp.tile([P, P], f32, space="PSUM")
                        nc.tensor.transpose(
                            out=tps[:], in_=aggn[:], identity=ident_sb[:]
                        )
                        aggnT = ntp.tile([P, P], f32, name="aggnT")
                        nc.vector.tensor_copy(out=aggnT[:], in_=tps[:])
                        ops = outpsp.tile([P, P], f32, space="PSUM")
                        nc.tensor.matmul(
                            out=ops[:],
                            lhsT=aggnT[:],
                            rhs=wbd_sb[:],
                            start=True,
                            stop=True,
                        )
                        osb = ntp.tile([P, P], f32, name="osb")
                        nc.vector.tensor_tensor(
                            out=osb[:],
                            in0=ops[:],
                            in1=biasrep_sb[:],
                            op=mybir.AluOpType.add,
                        )
                        nc.sync.dma_start(
                            out=out_dram[g, n_lo : n_lo + n_cnt, :],
                            in_=osb[0:n_cnt, :],
                        )
    nc.compile()
    return nc


def _host_inputs(x, adjacency, embedding, W, att_src, att_dst, bias, ep):
    """Build the per-core input maps (numpy only)."""
    NPAD = ((N + P - 1) // P) * P
    NB = NPAD // P
    emb = np.zeros((NPAD, C_IN), np.float32)
    emb[:N] = embedding
    embT = np.ascontiguousarray(emb.T)
    v_src = np.einsum("khc,hc->kh", W.reshape(C_IN, H, C_OUT), att_src)
    v_dst = np.einsum("khc,hc->kh", W.reshape(C_IN, H, C_OUT), att_dst)
    vboth = np.concatenate([v_src, v_dst], 1).astype(np.float32)  # [32, 8]
    wbd = np.zeros((P, P), np.float32)
    for h in range(H):
        wbd[h * C_IN : (h + 1) * C_IN, h * C_OUT : (h + 1) * C_OUT] = W[
            :, h * C_OUT : (h + 1) * C_OUT
        ]
    biasrep = np.broadcast_to(
        bias.astype(np.float32), (P, H * C_OUT)
    ).copy()
    iota = np.broadcast_to(np.arange(P, dtype=np.float32), (P, P)).copy()
    ident = np.eye(P, dtype=np.float32)

    xg_flat = x.reshape(G_TOT, N).astype(np.int64)
    in_maps = []
    for core in range(NCORES):
        xg = np.zeros((P, GPC, NB), np.int32)
        for g in range(GPC):
            xp = np.zeros(NPAD, np.int64)
            xp[:N] = xg_flat[core * GPC + g]
            xg[:, g, :] = xp.reshape(NB, P).T  # idx[p, i] = x[i*128+p]
        in_maps.append(
            {
                "emb": emb,
                "embT": embT,
                "vboth": vboth,
                "wbd": wbd,
                "biasrep": biasrep,
                "iota": iota,
                "ident": ident,
                "xg": xg,
                "srcidx": ep["src_idx"],
                "dstloc": ep["dstloc"],
            }
        )
    return in_maps


def kernel(x, adjacency, embedding, W, att_src, att_dst, bias):
    from concourse.bass_utils import run_bass_kernel_spmd

    x = np.asarray(x)
    adjacency = np.asarray(adjacency)
    embedding = np.asarray(embedding, np.float32)
    W = np.asarray(W, np.float32)
    att_src = np.asarray(att_src, np.float32)
    att_dst = np.asarray(att_dst, np.float32)
    bias = np.asarray(bias, np.float32)

    ep = _prep_edges(adjacency)
    nc = build_program(ep["nch"], ep["tiles"])
    in_maps = _host_inputs(
        x, adjacency, embedding, W, att_src, att_dst, bias, ep
    )
    import time as _time

    _t0 = _time.time()
    res = run_bass_kernel_spmd(
        nc, in_maps, core_ids=list(range(NCORES)), trace=False
    )
    kernel.last_exec_seconds = _time.time() - _t0
    outs = np.stack([r["out"] for r in res.results], 0)  # [8, 2, N, 128]
    full = outs.reshape(G_TOT, N, H * C_OUT)
    return full.reshape(B, DAYS, N * H * C_OUT)
